# revision 2
# baseline (speedup 1.0000x reference)
"""Trainium2 Bass kernel for nn_DetectionBEVLoss — slab clip, bf16, overlapped.

v3: channel-sliced DMA (geometry starts ~2us), focal Exp early / tree mid-
geometry, all Exp then all Ln (2 table loads saved), stt accum_out fusion for
every masked sum, YZ on ScalarE, in-place LO8/HI8. See kernel_v2 docstring for
the slab-clip math.
"""
import numpy as np

P = 128
S = 512
NPX = P * S
EPS = 1e-7

_CACHE = {}


def _ensure_ntff_hook():
    import sys, types
    if "antenv.axon_hooks" in sys.modules:
        return
    try:
        import trn_agent_boot.trn_boot as tb
        hook = tb._ntff_profile_via_ctypes('/opt/axon/libaxon_pjrt.so')
        mod = types.ModuleType("antenv.axon_hooks")
        mod.get_axon_ntff_profile_hook = lambda: hook
        sys.modules["antenv.axon_hooks"] = mod
    except Exception:
        pass


def _build(debug=False):
    import concourse.bacc as bacc
    import concourse.tile as tile
    import concourse.mybir as mybir
    import concourse.bass as bass

    F32 = mybir.dt.float32
    BF = mybir.dt.bfloat16
    I32 = mybir.dt.int32
    U8 = mybir.dt.uint8
    Alu = mybir.AluOpType
    Act = mybir.ActivationFunctionType
    AX_X = mybir.AxisListType.X

    nc = bacc.Bacc("TRN2", target_bir_lowering=False, debug=False, num_devices=8)

    for v in [float(np.pi / 2), 1.0]:
        t = nc.alloc_sbuf_tensor(f"const-f32-{v}", [P, 1], F32)
        nc.gpsimd.memset(t.ap(), v)
        nc.const_aps.aps[(F32, v)] = t.ap()
    nc.all_engine_barrier()

    d_cls = nc.dram_tensor("cls", [10, NPX], BF, kind="ExternalInput")
    d_rp = nc.dram_tensor("regp", [9, NPX], BF, kind="ExternalInput")
    d_rt = nc.dram_tensor("regt", [9, NPX], BF, kind="ExternalInput")
    d_ioup = nc.dram_tensor("ioup", [P, S], BF, kind="ExternalInput")
    d_iout = nc.dram_tensor("iout", [P, S], BF, kind="ExternalInput")
    d_ct = nc.dram_tensor("ct", [P, S], I32, kind="ExternalInput")
    d_w = nc.dram_tensor("w", [P, S], F32, kind="ExternalInput")
    d_out = nc.dram_tensor("out", [P, 8], F32, kind="ExternalOutput")

    V = nc.vector
    A_ = nc.scalar

    dbg_outs = []

    def dump(name, t):
        if not debug:
            return
        shp = [t.shape[0], int(np.prod(t.shape[1:]))]
        d = nc.dram_tensor(f"dbg_{name}", shp, t.dtype, kind="ExternalOutput")
        nc.sync.dma_start(out=d[:, :], in_=t)
        dbg_outs.append(name)

    def sl(t, i, k=1):
        return t[:, i * S:(i + k) * S]

    def apv(t, off, dims, inner=1):
        b = t[:, off * S:(off + 1) * S]
        ap = [b.ap[0]] + [[st * S, n] for (st, n) in dims] + [[1, inner * S]]
        assert len(ap) <= 3, f"AP too deep: {ap}"
        return bass.AP(tensor=b.tensor, offset=b.offset, ap=ap)

    def dmach(dst, dram, c0, k):
        nc.sync.dma_start(
            out=dst.rearrange("p (c f) -> p c f", c=k),
            in_=dram[c0:c0 + k, :].rearrange("c (p f) -> p c f", p=P))

    with tile.TileContext(nc) as tc:
      with tc.tile_pool(name="persist", bufs=1) as pp:
        OUT = pp.tile([P, 8], F32, name="OUT")
        ACCS = pp.tile([P, 8], F32, name="ACCS")
        SCR = pp.tile([P, S], BF, name="SCR")
        SCR2 = pp.tile([P, 2 * S], BF, name="SCR2")
        W = pp.tile([P, S], F32, name="W")
        WB = pp.tile([P, S], BF, name="WB")
        CTF = pp.tile([P, S], BF, name="CTF")
        CTI = pp.tile([P, S], I32, name="CTI")
        IOUP = pp.tile([P, S], BF, name="IOUP")
        IOUT = pp.tile([P, S], BF, name="IOUT")
        YAWS = pp.tile([P, 2 * S], BF, name="YAWS")     # (ya, yb)
        CENA = pp.tile([P, 2 * S], BF, name="CENA")     # (ax, ay)
        CENB = pp.tile([P, 2 * S], BF, name="CENB")
        DIMA = pp.tile([P, 2 * S], BF, name="DIMA")     # (lA, wA)
        DIMB = pp.tile([P, 2 * S], BF, name="DIMB")
        ZH_A = pp.tile([P, 2 * S], BF, name="ZH_A")     # (z, h) ch 2,5
        ZH_B = pp.tile([P, 2 * S], BF, name="ZH_B")
        VL_A = pp.tile([P, 2 * S], BF, name="VL_A")     # ch 7,8
        VL_B = pp.tile([P, 2 * S], BF, name="VL_B")
        PT = pp.tile([P, S], BF, name="PT")
        LG = pp.tile([P, S], BF, name="LG")
        OMP = pp.tile([P, S], BF, name="OMP")
        OM2 = pp.tile([P, S], BF, name="OM2")
        ALPH = pp.tile([P, S], BF, name="ALPH")
        AXB = pp.tile([P, S], BF, name="AXB")
        EB = pp.tile([P, S], BF, name="EB")
        SP = pp.tile([P, S], BF, name="SP")
        RL = pp.tile([P, S], BF, name="RL")

        nc.sync.dma_start(out=W, in_=d_w[:, :])
        dmach(sl(YAWS, 0), d_rp, 6, 1)
        dmach(sl(YAWS, 1), d_rt, 6, 1)
        dmach(DIMA, d_rp, 3, 2)
        dmach(DIMB, d_rt, 3, 2)
        dmach(CENA, d_rp, 0, 2)
        dmach(CENB, d_rt, 0, 2)
        nc.sync.dma_start(out=IOUP, in_=d_ioup[:, :])
        nc.sync.dma_start(out=IOUT, in_=d_iout[:, :])
        nc.sync.dma_start(out=CTI, in_=d_ct[:, :])
        dmach(sl(ZH_A, 0), d_rp, 2, 1)
        dmach(sl(ZH_A, 1), d_rp, 5, 1)
        dmach(sl(ZH_B, 0), d_rt, 2, 1)
        dmach(sl(ZH_B, 1), d_rt, 5, 1)
        dmach(VL_A, d_rp, 7, 2)
        dmach(VL_B, d_rt, 7, 2)

        V.tensor_reduce(ACCS[:, 6:7], W, AX_X, Alu.add)
        V.tensor_copy(WB, W)
        V.tensor_copy(CTF, CTI)
        A_.activation(AXB, IOUP, Act.Abs)

        with tc.tile_pool(name="geo", bufs=1) as pg:
            # ---- trig (set #1) ----
            TRIG = pg.tile([P, 4 * S], BF, name="TRIG")   # (sa, ca, sb, cb)
            A_.activation(sl(TRIG, 0), sl(YAWS, 0), Act.Sin)
            A_.activation(sl(TRIG, 1), sl(YAWS, 0), Act.Sin, bias=float(np.pi / 2))
            A_.activation(sl(TRIG, 2), sl(YAWS, 1), Act.Sin)
            A_.activation(sl(TRIG, 3), sl(YAWS, 1), Act.Sin, bias=float(np.pi / 2))
            sa, ca, sb, cb = sl(TRIG, 0), sl(TRIG, 1), sl(TRIG, 2), sl(TRIG, 3)

            HD = pg.tile([P, 4 * S], BF, name="HD")       # half dims
            V.tensor_scalar(HD[:, 0:2 * S], DIMA, 0.5, None, Alu.mult)
            V.tensor_scalar(HD[:, 2 * S:4 * S], DIMB, 0.5, None, Alu.mult)
            HLA, HWA, HLB, HWB = sl(HD, 0), sl(HD, 1), sl(HD, 2), sl(HD, 3)

            C32 = pg.tile([P, 4 * S], F32, name="C32")
            R32b = pg.tile([P, 4 * S], F32, name="R32b")
            V.tensor_scalar(C32[:, 0:2 * S], DIMA, EPS, None, Alu.max)
            V.tensor_scalar(C32[:, 2 * S:4 * S], DIMB, EPS, None, Alu.max)
            V.reciprocal_approx_fast(R32b, C32)
            RD = pg.tile([P, 4 * S], BF, name="RD")       # (rlA, rwA, rlB, rwB)
            V.tensor_copy(RD, R32b)
            rlA, rwA, rlB, rwB = sl(RD, 0), sl(RD, 1), sl(RD, 2), sl(RD, 3)

            WLLWa = pg.tile([P, 2 * S], BF, name="WLLWa")
            WLLWb = pg.tile([P, 2 * S], BF, name="WLLWb")
            V.tensor_tensor(sl(WLLWa, 0), sl(DIMA, 1), rlA, Alu.mult)
            V.tensor_tensor(sl(WLLWa, 1), sl(DIMA, 0), rwA, Alu.mult)
            V.tensor_tensor(sl(WLLWb, 0), sl(DIMB, 1), rlB, Alu.mult)
            V.tensor_tensor(sl(WLLWb, 1), sl(DIMB, 0), rwB, Alu.mult)

            # ---- arctan v-term (still trig set) ----
            ATD = pg.tile([P, S], BF, name="ATD")
            Vv = pg.tile([P, S], BF, name="Vv")
            for i, WLLW in enumerate((WLLWa, WLLWb)):
                MN = pg.tile([P, S], BF, name=f"MN{i}", tag="s1")
                V.tensor_tensor(MN, sl(WLLW, 0), sl(WLLW, 1), Alu.min)
                AA = pg.tile([P, S], BF, name=f"AA{i}", tag="s2")
                A_.activation(AA, MN, Act.Arctan)
                TTb = pg.tile([P, S], BF, name=f"TTb{i}", tag="s3")
                V.tensor_scalar(TTb, AA, -2.0, float(np.pi / 2), Alu.mult, Alu.add)
                M8 = pg.tile([P, S], U8, name=f"M8{i}", tag="m8")
                V.tensor_scalar(M8, sl(WLLW, 0), 1.0, None, Alu.is_gt)
                MF = pg.tile([P, S], BF, name=f"MF{i}", tag="s4")
                V.tensor_copy(MF, M8)
                V.tensor_tensor(TTb, TTb, MF, Alu.mult)
                V.tensor_tensor(AA, AA, TTb, Alu.add)
                if i == 0:
                    V.tensor_copy(ATD, AA)
                else:
                    V.tensor_tensor(ATD, ATD, AA, Alu.subtract)
            A_.activation(Vv, ATD, Act.Square, scale=float(2.0 / np.pi))

            # ================= focal front (set #2: exp) =================
            with tc.tile_pool(name="focal", bufs=1) as pf:
                CLS = pf.tile([P, 10 * S], BF, name="CLS")
                nc.sync.dma_start(out=CLS.rearrange("p (c f) -> p c f", c=10),
                                  in_=d_cls[:, :].rearrange("c (p f) -> p c f", p=P))
                A_.activation(CLS, CLS, Act.Exp)          # E in place
                A_.activation(EB, AXB, Act.Exp, scale=-1.0)
                SC = pf.tile([P, 5 * S], BF, name="SC")
                V.tensor_tensor(SC, CLS[:, 0:5 * S], CLS[:, 5 * S:10 * S], Alu.add)
                V.tensor_tensor(SC[:, 0:2 * S], SC[:, 0:2 * S], SC[:, 2 * S:4 * S],
                                Alu.add)
                V.tensor_tensor(sl(SC, 0), sl(SC, 0), sl(SC, 1), Alu.add)
                Ssum = pf.tile([P, S], BF, name="Ssum")
                V.tensor_tensor(Ssum, sl(SC, 0), sl(SC, 4), Alu.add)
                ET = pf.tile([P, S], BF, name="ET")
                V.tensor_copy(ET, sl(CLS, 0))
                for c in range(1, 10):
                    MC = pf.tile([P, S], U8, name=f"MC_{c}", tag="MC")
                    V.tensor_scalar(MC, CTF, float(c), None, Alu.is_equal)
                    V.copy_predicated(ET, MC, sl(CLS, c))
                S32 = pf.tile([P, S], F32, name="S32")
                R32f = pf.tile([P, S], F32, name="R32f")
                V.tensor_copy(S32, Ssum)
                V.reciprocal_approx_fast(R32f, S32)
                RSb = pf.tile([P, S], BF, name="RSb")
                V.tensor_copy(RSb, R32f)
                V.tensor_tensor(PT, ET, RSb, Alu.mult)
                V.tensor_scalar(PT, PT, EPS, 1.0 - EPS, Alu.max, Alu.min)
                V.tensor_scalar(OMP, PT, -1.0, 1.0, Alu.mult, Alu.add)
                # BCE partials (no SP dependency yet)
                V.tensor_scalar(RL, IOUP, 0.0, None, Alu.max)
                V.tensor_tensor(SCR, IOUP, IOUT, Alu.mult)
                V.tensor_tensor(RL, RL, SCR, Alu.subtract)

            # ---- ln batch (set #3) + universal funcs ----
            A_.activation(LG, PT, Act.Ln)
            A_.activation(SP, EB, Act.Ln, bias=1.0)
            A_.activation(OM2, OMP, Act.Square)

            # ---- geometry core ----
            CS = pg.tile([P, 2 * S], BF, name="CS")
            Cc, Sn = sl(CS, 0), sl(CS, 1)
            T1 = pg.tile([P, S], BF, name="T1g")
            T2 = pg.tile([P, S], BF, name="T2g")
            V.tensor_tensor(T1, ca, cb, Alu.mult)
            V.tensor_tensor(T2, sa, sb, Alu.mult)
            V.tensor_tensor(Cc, T1, T2, Alu.add)
            V.tensor_tensor(T1, sa, cb, Alu.mult)
            V.tensor_tensor(T2, ca, sb, Alu.mult)
            V.tensor_tensor(Sn, T1, T2, Alu.subtract)
            DTINY = pg.tile([P, S], BF, name="DTINY")
            V.memset(DTINY, 1e-20)
            MZ = pg.tile([P, S], U8, name="MZ", tag="m8")
            V.tensor_scalar(MZ, Sn, 0.0, None, Alu.is_equal)
            V.copy_predicated(Sn, MZ, DTINY)
            CS32 = pg.tile([P, 2 * S], F32, name="CS32")
            RCS32 = pg.tile([P, 2 * S], F32, name="RCS32")
            V.tensor_copy(CS32, CS)
            V.reciprocal_approx_fast(RCS32, CS32)
            RCS = pg.tile([P, 2 * S], BF, name="RCS")
            V.tensor_copy(RCS, RCS32)
            rC, rS = sl(RCS, 0), sl(RCS, 1)

            TT0 = pg.tile([P, 2 * S], BF, name="TT0")     # (T, Tin)
            V.tensor_tensor(sl(TT0, 0), Sn, rC, Alu.mult)
            V.tensor_tensor(sl(TT0, 1), Cc, rS, Alu.mult)

            DXY = pg.tile([P, 2 * S], BF, name="DXY")
            V.tensor_tensor(DXY, CENA, CENB, Alu.subtract)
            dx, dy = sl(DXY, 0), sl(DXY, 1)
            G = pg.tile([P, 4 * S], BF, name="G")
            P2a = pg.tile([P, 2 * S], BF, name="P2a", tag="p2a")
            P2b = pg.tile([P, 2 * S], BF, name="P2b", tag="p2b")
            for i, (c_, s_) in enumerate(((cb, sb), (ca, sa))):
                # P2a = (c*dx, c*dy), P2b = (s*dx, s*dy)
                V.tensor_tensor(P2a, apv(TRIG, 3 - 2 * i, [(0, 2)]), DXY, Alu.mult)
                V.tensor_tensor(P2b, apv(TRIG, 2 - 2 * i, [(0, 2)]), DXY, Alu.mult)
                V.tensor_tensor(sl(G, 2 * i), sl(P2a, 0), sl(P2b, 1), Alu.add)
                V.tensor_tensor(sl(G, 2 * i + 1), sl(P2a, 1), sl(P2b, 0),
                                Alu.subtract)

            # focal + bce finishers (ScalarE long done by now)
            FLt = pg.tile([P, S], BF, name="FLt", tag="s1")
            MPOS = pg.tile([P, S], U8, name="MPOS", tag="m8")
            V.tensor_scalar(MPOS, CTF, 0.0, None, Alu.is_gt)
            V.tensor_copy(ALPH, MPOS)
            V.tensor_scalar(ALPH, ALPH, -0.5, 0.75, Alu.mult, Alu.add)
            V.tensor_tensor(FLt, OM2, LG, Alu.mult)
            V.scalar_tensor_tensor(SCR, FLt, -1.0, ALPH, Alu.mult, Alu.mult,
                                   accum_out=ACCS[:, 0:1])
            V.tensor_tensor(RL, RL, SP, Alu.add)
            V.scalar_tensor_tensor(SCR, RL, 1.0, WB, Alu.mult, Alu.mult,
                                   accum_out=ACCS[:, 5:6])

            # ---- d2 / c2 ----
            ABS4 = pg.tile([P, 4 * S], BF, name="ABS4")
            A_.activation(ABS4, TRIG, Act.Abs)            # |sa|,|ca|,|sb|,|cb|
            EXA = pg.tile([P, 2 * S], BF, name="EXA")
            EXB = pg.tile([P, 2 * S], BF, name="EXB")
            for i, (EX_, hl, hw, ac, as_) in enumerate((
                    (EXA, HLA, HWA, sl(ABS4, 1), sl(ABS4, 0)),
                    (EXB, HLB, HWB, sl(ABS4, 3), sl(ABS4, 2)))):
                V.tensor_tensor(T1, hl, ac, Alu.mult)
                V.tensor_tensor(T2, hw, as_, Alu.mult)
                V.tensor_tensor(sl(EX_, 0), T1, T2, Alu.add)   # Ex
                V.tensor_tensor(T1, hl, as_, Alu.mult)
                V.tensor_tensor(T2, hw, ac, Alu.mult)
                V.tensor_tensor(sl(EX_, 1), T1, T2, Alu.add)   # Ey
            PMAX = pg.tile([P, 2 * S], BF, name="PMAX")
            PMIN = pg.tile([P, 2 * S], BF, name="PMIN")
            Q1t = pg.tile([P, 2 * S], BF, name="Q1t")
            Q2t = pg.tile([P, 2 * S], BF, name="Q2t")
            V.tensor_tensor(Q1t, CENA, EXA, Alu.add)
            V.tensor_tensor(Q2t, CENB, EXB, Alu.add)
            V.tensor_tensor(PMAX, Q1t, Q2t, Alu.max)
            V.tensor_tensor(Q1t, CENA, EXA, Alu.subtract)
            V.tensor_tensor(Q2t, CENB, EXB, Alu.subtract)
            V.tensor_tensor(PMIN, Q1t, Q2t, Alu.min)
            BWH = pg.tile([P, 2 * S], BF, name="BWH")
            V.tensor_tensor(BWH, PMAX, PMIN, Alu.subtract)
            SQ2 = pg.tile([P, 2 * S], BF, name="SQ2")
            A_.activation(SQ2, BWH, Act.Square)
            C232 = pg.tile([P, S], F32, name="C232")
            V.tensor_tensor(C232, sl(SQ2, 0), sl(SQ2, 1), Alu.add)
            V.tensor_scalar(C232, C232, EPS, None, Alu.max)
            R32s = pg.tile([P, S], F32, name="R32s")
            V.reciprocal_approx_fast(R32s, C232)
            RC2 = pg.tile([P, S], BF, name="RC2")
            V.tensor_copy(RC2, R32s)
            DXY2 = pg.tile([P, 2 * S], BF, name="DXY2", tag="s2")
            V.tensor_tensor(DXY2, DXY, DXY, Alu.mult)
            D2C2 = pg.tile([P, S], BF, name="D2C2")
            V.tensor_tensor(D2C2, sl(DXY2, 0), sl(DXY2, 1), Alu.add)
            V.tensor_tensor(D2C2, D2C2, RC2, Alu.mult)

            # ================= slab clip passes =================
            DT6 = pg.tile([P, 6 * S], BF, name="DT6")
            KT = pg.tile([P, 6 * S], BF, name="KT")
            with tc.tile_pool(name="clip", bufs=1) as pc:
                DTs = []
                for pi in range(2):
                    if pi == 0:
                        rl_, rw_ = rlA, rwA
                        hL, hW = HLB, HWB
                        G1, G2 = sl(G, 0), sl(G, 1)
                        gsw = False
                        WLLW = WLLWa
                        yzsign = (0.5, -0.5, -0.5, 0.5)
                        cxy = CENA
                        hl_s, hw_s = HLA, HWA
                    else:
                        rl_, rw_ = rlB, rwB
                        hL, hW = HLA, HWA
                        G1, G2 = sl(G, 2), sl(G, 3)
                        gsw = True
                        WLLW = WLLWb
                        yzsign = (-0.5, 0.5, 0.5, -0.5)
                        cxy = CENB
                        hl_s, hw_s = HLB, HWB

                    R4 = pc.tile([P, 4 * S], BF, name=f"R4_{pi}", tag="R4")
                    V.tensor_tensor(sl(R4, 0), rl_, rC, Alu.mult)
                    if pi == 0:
                        V.scalar_tensor_tensor(sl(R4, 1), rw_, -1.0, rS,
                                               Alu.mult, Alu.mult)
                        V.tensor_tensor(sl(R4, 2), rl_, rS, Alu.mult)
                    else:
                        V.tensor_tensor(sl(R4, 1), rw_, rS, Alu.mult)
                        V.scalar_tensor_tensor(sl(R4, 2), rl_, -1.0, rS,
                                               Alu.mult, Alu.mult)
                    V.tensor_tensor(sl(R4, 3), rw_, rC, Alu.mult)

                    HG = pc.tile([P, 4 * S], BF, name=f"HG_{pi}", tag="HG")
                    o1, o2 = (Alu.subtract, Alu.add) if gsw else (Alu.add, Alu.subtract)
                    V.tensor_tensor(sl(HG, 0), hL, G1, o1)
                    V.tensor_tensor(sl(HG, 1), hL, G1, o2)
                    V.tensor_tensor(sl(HG, 2), hW, G2, o1)
                    V.tensor_tensor(sl(HG, 3), hW, G2, o2)

                    XT = pc.tile([P, 8 * S], BF, name=f"XT_{pi}", tag="XT")
                    for sb_ in range(2):
                        hg2 = HG[:, 2 * sb_ * S:(2 * sb_ + 2) * S]
                        V.tensor_tensor(XT[:, (4 * sb_) * S:(4 * sb_ + 2) * S],
                                        hg2, apv(R4, 2 * sb_, [(0, 2)]), Alu.mult)
                        V.tensor_tensor(XT[:, (4 * sb_ + 2) * S:(4 * sb_ + 4) * S],
                                        hg2, apv(R4, 2 * sb_ + 1, [(0, 2)]), Alu.mult)

                    MNN = pc.tile([P, 8 * S], BF, name=f"MNN_{pi}", tag="MNN")
                    HNN = pc.tile([P, 8 * S], BF, name=f"HNN_{pi}", tag="HNN")
                    for sb_ in range(2):
                        x1 = apv(XT, 4 * sb_, [(2, 2)])
                        x2 = apv(XT, 4 * sb_ + 1, [(2, 2)])
                        o_ = 4 * sb_ * S
                        V.scalar_tensor_tensor(MNN[:, o_:o_ + 2 * S],
                                               x2, -1.0, x1, Alu.mult, Alu.min)
                        V.scalar_tensor_tensor(MNN[:, o_ + 2 * S:o_ + 4 * S],
                                               x1, -1.0, x2, Alu.mult, Alu.min)
                        V.scalar_tensor_tensor(HNN[:, o_:o_ + 2 * S],
                                               x2, -1.0, x1, Alu.mult, Alu.max)
                        V.scalar_tensor_tensor(HNN[:, o_ + 2 * S:o_ + 4 * S],
                                               x1, -1.0, x2, Alu.mult, Alu.max)

                    U4 = pc.tile([P, 4 * S], BF, name=f"U4_{pi}", tag="U4")
                    V.tensor_tensor(U4[:, 0:2 * S], WLLW,
                                    apv(TT0, 0, [(0, 2)]), Alu.mult)
                    V.tensor_tensor(U4[:, 2 * S:4 * S], WLLW,
                                    apv(TT0, 1, [(0, 2)]), Alu.mult)
                    YZ = pc.tile([P, 4 * S], BF, name=f"YZ_{pi}", tag=f"YZ{pi}")
                    for (dst, src) in ((0, 0), (1, 3), (2, 2), (3, 1)):
                        A_.activation(sl(YZ, dst), sl(U4, src), Act.Copy,
                                      bias=0.5, scale=yzsign[dst])

                    # lo8/hi8 in place over MNN/HNN
                    for sb_ in range(2):
                        yzv = apv(YZ, 2 * sb_, [(0, 2)], inner=2)
                        o_ = 4 * sb_ * S
                        V.tensor_tensor(MNN[:, o_:o_ + 4 * S],
                                        MNN[:, o_:o_ + 4 * S], yzv, Alu.add)
                        V.tensor_tensor(HNN[:, o_:o_ + 4 * S],
                                        HNN[:, o_:o_ + 4 * S], yzv, Alu.add)
                    # reuse dead buffers: LO4/HI4 <- XT halves, R0 <- HG,
                    # R1 <- R4, SS <- U4, DT <- YZ
                    LO4 = XT[:, 0:4 * S]
                    HI4 = XT[:, 4 * S:8 * S]
                    V.tensor_tensor(LO4, MNN[:, 0:4 * S], MNN[:, 4 * S:8 * S],
                                    Alu.max)
                    V.tensor_tensor(HI4, HNN[:, 0:4 * S], HNN[:, 4 * S:8 * S],
                                    Alu.min)

                    R0 = HG[:, 0:4 * S]
                    R1 = R4[:, 0:4 * S]
                    A_.activation(R0, LO4, Act.Relu)
                    A_.activation(R1, HI4, Act.Relu, scale=-1.0, bias=1.0)
                    SS = U4[:, 0:4 * S]
                    V.tensor_tensor(SS, R0, R1, Alu.add)
                    DT = YZ[:, 0:4 * S]
                    A_.activation(DT, SS, Act.Relu, scale=-1.0, bias=1.0)

                    # filler while ScalarE runs the relu chain
                    if pi == 0:
                        for (zh, col) in ((0, 2), (1, 3)):
                            V.tensor_tensor(SCR, sl(ZH_A, zh), sl(ZH_B, zh),
                                            Alu.subtract)
                            V.tensor_tensor(SCR, SCR, WB, Alu.mult)
                            V.scalar_tensor_tensor(SCR, SCR, 0.5, SCR, Alu.mult,
                                                   Alu.mult,
                                                   accum_out=ACCS[:, col:col + 1])
                    else:
                        V.tensor_tensor(SCR2, VL_A, VL_B, Alu.subtract)
                        V.tensor_tensor(SCR2, SCR2, apv(WB, 0, [(0, 2)]), Alu.mult)
                        V.scalar_tensor_tensor(SCR2, SCR2, 0.5, SCR2, Alu.mult,
                                               Alu.mult, accum_out=ACCS[:, 4:5])

                    # KT partials don't depend on DT — emit while ScalarE
                    # runs the relu chain. P2a=(cx*s,cy*s), P2b=(cx*c,cy*c).
                    V.scalar_tensor_tensor(sl(KT, 3 * pi), hl_s, -2.0, hw_s,
                                           Alu.mult, Alu.mult)
                    V.tensor_tensor(P2a, apv(TRIG, 2 * pi, [(0, 2)]), cxy,
                                    Alu.mult)
                    V.tensor_tensor(P2b, apv(TRIG, 2 * pi + 1, [(0, 2)]), cxy,
                                    Alu.mult)
                    V.tensor_tensor(T1, sl(P2a, 0), sl(P2b, 1), Alu.subtract)
                    V.scalar_tensor_tensor(sl(KT, 3 * pi + 1), hl_s, 2.0, T1,
                                           Alu.mult, Alu.mult)
                    V.tensor_tensor(T2, sl(P2b, 0), sl(P2a, 1), Alu.add)
                    V.scalar_tensor_tensor(sl(KT, 3 * pi + 2), hw_s, 2.0, T2,
                                           Alu.mult, Alu.mult)
                    DTs.append(DT)

                # deferred dt contractions: pass-0's relu chain overlapped
                # pass-1's DVE front; both DTs are live (per-pass YZ tags)
                for pi in range(2):
                    DT = DTs[pi]
                    Q2 = SCR2
                    V.tensor_tensor(Q2, DT[:, 0:2 * S], DT[:, 2 * S:4 * S], Alu.add)
                    V.tensor_tensor(sl(DT6, 3 * pi), sl(Q2, 0), sl(Q2, 1), Alu.add)
                    V.tensor_tensor(DT6[:, (3 * pi + 1) * S:(3 * pi + 3) * S],
                                    DT[:, 2 * S:4 * S], DT[:, 0:2 * S], Alu.subtract)

            # ---- assemble iou + bev ----
            SROD = pg.tile([P, 6 * S], BF, name="SROD")
            V.tensor_tensor(SROD, KT, DT6, Alu.mult)
            F3 = pg.tile([P, 3 * S], BF, name="F3")
            V.tensor_tensor(F3, SROD[:, 0:3 * S], SROD[:, 3 * S:6 * S], Alu.add)
            SAB = pg.tile([P, S], BF, name="SAB")
            V.tensor_tensor(SAB, sl(F3, 0), sl(F3, 1), Alu.add)
            V.tensor_tensor(SAB, SAB, sl(F3, 2), Alu.add)

            IB = pg.tile([P, S], BF, name="IB")
            A_.activation(IB, SAB, Act.Abs, scale=0.5)
            UN32 = pg.tile([P, S], F32, name="UN32")
            V.tensor_tensor(T1, sl(KT, 0), sl(KT, 3), Alu.add)
            V.scalar_tensor_tensor(T1, T1, -2.0, IB, Alu.mult, Alu.subtract)
            V.tensor_scalar(UN32, T1, EPS, None, Alu.max)
            V.reciprocal_approx_fast(R32s, UN32)
            IOU = pg.tile([P, S], BF, name="IOU")
            V.tensor_copy(T2, R32s)
            V.tensor_tensor(IOU, IB, T2, Alu.mult)
            dump("IOU", IOU)

            DEN32 = pg.tile([P, S], F32, name="DEN32")
            V.tensor_scalar(T1, Vv, 1.0, float(1.0 + EPS), Alu.mult, Alu.add)
            V.tensor_tensor(T1, T1, IOU, Alu.subtract)
            V.tensor_copy(DEN32, T1)
            V.reciprocal_approx_fast(R32s, DEN32)
            V.tensor_copy(T2, R32s)
            ALC = pg.tile([P, S], BF, name="ALC", tag="s3")
            V.tensor_tensor(ALC, Vv, T2, Alu.mult)
            V.tensor_tensor(ALC, ALC, Vv, Alu.mult)
            LB = pg.tile([P, S], BF, name="LB", tag="s4")
            V.tensor_scalar(LB, IOU, -1.0, 1.0, Alu.mult, Alu.add)
            V.tensor_tensor(LB, LB, D2C2, Alu.add)
            V.tensor_tensor(LB, LB, ALC, Alu.add)
            V.scalar_tensor_tensor(SCR, LB, 1.0, WB, Alu.mult, Alu.mult,
                                   accum_out=ACCS[:, 1:2])
            dump("accs", ACCS)

        A_.memzero(ACCS[:, 7:8])
        V.tensor_copy(OUT, ACCS)
        nc.sync.dma_start(out=d_out[:, :], in_=OUT)

    nc.compile()
    nc._dbg_names = dbg_outs
    return nc


def _get_nc():
    if "nc" not in _CACHE:
        _ensure_ntff_hook()
        _CACHE["nc"] = _build()
    return _CACHE["nc"]


def make_in_maps(inputs):
    import ml_dtypes
    bf16 = ml_dtypes.bfloat16
    cls_pred = np.asarray(inputs["cls_pred"], dtype=np.float32)
    reg_pred = np.asarray(inputs["reg_pred"], dtype=np.float32)
    iou_pred = np.asarray(inputs["iou_pred"], dtype=np.float32)
    cls_targets = np.asarray(inputs["cls_targets"], dtype=np.int32)
    reg_targets = np.asarray(inputs["reg_targets"], dtype=np.float32)
    reg_weights = np.asarray(inputs["reg_weights"], dtype=np.float32)
    iou_targets = np.asarray(inputs["iou_targets"], dtype=np.float32)
    B = cls_pred.shape[0]
    in_maps = []
    for b in range(B):
        in_maps.append({
            "cls": np.ascontiguousarray(cls_pred[b].reshape(10, NPX)).astype(bf16),
            "regp": np.ascontiguousarray(reg_pred[b].reshape(9, NPX)).astype(bf16),
            "regt": np.ascontiguousarray(reg_targets[b].reshape(9, NPX)).astype(bf16),
            "ioup": np.ascontiguousarray(iou_pred[b].reshape(P, S)).astype(bf16),
            "iout": np.ascontiguousarray(iou_targets[b].reshape(P, S)).astype(bf16),
            "ct": np.ascontiguousarray(cls_targets[b].reshape(P, S)),
            "w": np.ascontiguousarray(reg_weights[b].reshape(P, S)),
        })
    return in_maps


def kernel(**inputs):
    from concourse.bass_utils import run_bass_kernel_spmd

    nc = _get_nc()
    in_maps = make_in_maps(inputs)
    res = run_bass_kernel_spmd(nc, in_maps, core_ids=list(range(8)))
    _CACHE["last_result"] = res
    sums = np.zeros(8, np.float64)
    for r in res.results:
        sums += r["out"].astype(np.float64).sum(axis=0)
    num_pos = max(sums[6], 1.0)
    out = np.array([sums[0], sums[1], sums[2], sums[3], sums[4], sums[5]],
                   np.float64) / num_pos
    return out.astype(np.float32)
